# revision 17
# baseline (speedup 1.0000x reference)
"""DCP pooling kernel for Trainium2 (8 NeuronCores, data-parallel over batch).

Math: reference pads x spatially, takes |min over channels| of the padded
image, sums all 3x3 sliding windows, then sums everything.  Padded zeros
contribute nothing, so the result collapses to

    sum_{b,h,w} |min_c x[b,c,h,w]| * rw(h) * cw(w)

with rw(h) = 2 if h in {0, H-1} else 3 (same for cw).  Pure streaming
reduction: read 192 MiB, emit one scalar -> memory-bound.

Layout: each 1024x1024 channel plane is viewed per half as [128, 4096]
(partition p holds 4 consecutive rows: image row = 512*h + 4*p + q,
flat col = q*1024 + w).  Every DMA descriptor is then 8KB of contiguous
DRAM per partition - measured ~408 GB/s per core vs ~344 GB/s for an
interleaved-channel 4KB-descriptor layout.

Device program per core (2 images = 4 half-planes = "chunks"):
  sync  (SP):   HWDGE loads, [128,2048] col-tiles x 3 channels per tile;
                the last half-plane is split into 6 shrinking col-pieces
                so the post-stream compute tail is tiny.  One shared load
                semaphore; consumers use cumulative per-load thresholds
                (min over channels starts when c0+c1 have landed).
  vector(DVE):  channel-min per tile (two tensor_tensor.min); edge-column
                extractions (|col 0| / |col 1023| per row-group) for
                chunks 0,1 and the pieces; last piece's rowsum.
  scalar(ACT):  Abs activation with fused accum_out per row-group slice
                -> per-(p,q)-rowsum staging cols; chunk 2's edge columns;
                issues the final [128,47] staging DMA to DRAM.
Host: applies the 2-vs-3 row/col weights in float64 from the staging
columns (rows 0/1023 get dedicated q-slices so their row sums are exact).
"""

import numpy as np

import concourse.bass as bass
import concourse.bacc as bacc
import concourse.mybir as mybir
from concourse.alu_op_type import AluOpType
from concourse.bass_utils import run_bass_kernel_spmd
from contextlib import ExitStack

B = 16            # full batch
NCORES = 8
BPC = B // NCORES  # images per core
C = 3
H = W = 1024
P = 128
F = 4096          # flat cols per half-plane: q*1024 + w, q = 0..3
f32 = mybir.dt.float32

# last half-plane (b=1, h=1) piece split (cols)
PIECES = [(0, 1024), (1024, 2048), (2048, 3072),
          (3072, 3584), (3584, 3840), (3840, 4096)]

# staging columns
# 0..13: ACT rowsum cols (per tile / piece, see build)
RSP5 = 14              # DVE rowsum of [3840:4096] of last half-plane
EDGE0 = 15             # chunk0 edges (DVE): q0c0,q0c1,q1c0,q1c1,q2c0,...
EDGE1 = 23             # chunk1 edges (DVE)
EDGE2 = 31             # chunk2 edges (ACT)
PEDGE = 39             # piece edges (DVE): q0c0,q0c1,q1c0,q1c1,q2c0,q2c1
Q3C0 = 45              # piece q3 col-0 edge (DVE)
Q3C1 = 46              # last piece's col-1023 edge (DVE)
NCOLS = 47
# staging/out padded to 128 cols: 512B per partition keeps the final DMA's
# descriptors at the SDMA line-rate threshold (no DRAM read-modify-write)
OUTCOLS = 128

_CACHE: dict = {}


def build_nc() -> bass.Bass:
    nc = bacc.Bacc(detect_race_conditions=False)
    x = nc.declare_dram_parameter("x", [BPC, C, 2, P, F], f32, isOutput=False)
    out = nc.declare_dram_parameter("out", [P, OUTCOLS], f32, isOutput=True)

    # full tiles: (b, h, col range); chunk = 2*b + h; pieces cover (1,1)
    fulls = [(b, h, c0, c0 + 2048)
             for (b, h) in [(0, 0), (0, 1), (1, 0)] for c0 in (0, 2048)]

    with ExitStack() as ctx:
        ec = ctx.enter_context
        # 3 rotating trio slots for full tiles, [128, 3*2048] each
        slots = ec(nc.sbuf_tensor("slots", [P, 3 * 3 * 2048], f32))
        # pieces trio buffer for the (1,1) half-plane, [128, 3*4096]
        pslot = ec(nc.sbuf_tensor("pslot", [P, 3 * F], f32))
        t1 = ec(nc.sbuf_tensor("t1", [P, F], f32))
        m2a = ec(nc.sbuf_tensor("m2a", [P, F], f32))   # chunks 0, 2
        m2b = ec(nc.sbuf_tensor("m2b", [P, F], f32))   # chunk 1
        m2c = ec(nc.sbuf_tensor("m2c", [P, F], f32))   # chunk 3 (pieces)
        absout = ec(nc.sbuf_tensor("absout", [P, 2048], f32))
        stag = ec(nc.sbuf_tensor("stag", [P, OUTCOLS], f32))
        zbias = ec(nc.sbuf_tensor("zbias", [P, 1], f32))
        acksink = ec(nc.sbuf_tensor("acksink", [P, 1], f32))

        # Per-tile trio semaphores.  A wait threshold on a DMA semaphore is
        # only exact when it equals ALL increments ever issued on it (48 =
        # 16 SDMA engines x 3 channel loads) -- cumulative thresholds on a
        # shared semaphore raced (individual engines can lag a full load
        # behind the aggregate count).
        csem = [ec(nc.semaphore(f"csem{k}")) for k in range(12)]
        min2_done = ec(nc.semaphore("min2_done"))
        act_done = ec(nc.semaphore("act_done"))
        act_fin = ec(nc.semaphore("act_fin"))
        dve_fin = ec(nc.semaphore("dve_fin"))
        osem = ec(nc.semaphore("osem"))
        block = ec(nc.Block(no_gpsimd_drain=True))

        def slot_ap(k, cols):
            base = (k % 3) * 3 * 2048
            return [slots[:, base + c * 2048:base + c * 2048 + cols]
                    for c in range(C)]

        m2_of = [m2a, m2b, m2a]  # per chunk 0..2

        @block.sync
        def _(sync):
            for k, (b, h, c0, c1) in enumerate(fulls):
                if k >= 3:
                    # trio slot k%3 reused from tile k-3; freed by its min2
                    sync.wait_ge(min2_done, k - 2)
                dsts = slot_ap(k, c1 - c0)
                for c in range(C):
                    sync.dma_start(out=dsts[c], in_=x[b, c, h][:, c0:c1]
                                   ).then_inc(csem[k], 16)
            for j, (c0, c1) in enumerate(PIECES):
                for c in range(C):
                    sync.dma_start(out=pslot[:, c * F + c0:c * F + c1],
                                   in_=x[1, c, 1][:, c0:c1]
                                   ).then_inc(csem[6 + j], 16)
            # final staging store: sync's HWDGE queue is warm (36 loads);
            # issuing from ACT's cold queue cost ~0.5us extra fetch latency
            sync.wait_ge(act_fin, 1)
            sync.wait_ge(dve_fin, 1)
            sync.dma_start(out=out[:], in_=stag[:]).then_inc(osem, 16)
            sync.wait_ge(osem, 16)

        @block.vector
        def _(vector):
            # zbias for ACT's Abs: ACT's first activation waits min2_done>=1,
            # which transitively orders it after this DVE memset.
            vector.memset(zbias[:], 0.0)

            def edge(col, m2, fc):
                vector.tensor_reduce(stag[:, col:col + 1], m2[:, fc:fc + 1],
                                     mybir.AxisListType.X, AluOpType.add,
                                     apply_absolute_value=True)

            for k, (b, h, c0, c1) in enumerate(fulls):
                chunk = 2 * b + h
                m2 = m2_of[chunk]
                s0, s1, s2 = slot_ap(k, c1 - c0)
                vector.wait_ge(csem[k], 48)
                vector.tensor_tensor(t1[:, c0:c1], s0, s1, AluOpType.min)
                if k == 4:
                    # m2a reuse: chunk-0 rowsum activations must have read it
                    vector.wait_ge(act_done, 3)
                vector.tensor_tensor(m2[:, c0:c1], t1[:, c0:c1], s2,
                                     AluOpType.min).then_inc(min2_done, 1)
                if chunk < 2:
                    # 2 q-groups per 2048-col tile -> 4 edge columns
                    base = EDGE0 if chunk == 0 else EDGE1
                    qbase = 0 if c0 == 0 else 2
                    for qq in range(2):
                        q = qbase + qq
                        edge(base + 2 * q, m2, q * 1024)
                        edge(base + 2 * q + 1, m2, q * 1024 + 1023)

            for j, (c0, c1) in enumerate(PIECES):
                vector.wait_ge(csem[6 + j], 48)
                vector.tensor_tensor(t1[:, c0:c1], pslot[:, c0:c1],
                                     pslot[:, F + c0:F + c1], AluOpType.min)
                vector.tensor_tensor(m2c[:, c0:c1], t1[:, c0:c1],
                                     pslot[:, 2 * F + c0:2 * F + c1],
                                     AluOpType.min).then_inc(min2_done, 1)
                if j < 3:
                    # pieces 0-2 are whole q-groups: both edge columns
                    edge(PEDGE + 2 * j, m2c, c0)
                    edge(PEDGE + 2 * j + 1, m2c, c0 + 1023)
                elif j == 3:
                    edge(Q3C0, m2c, 3072)
            # last piece epilogue: its rowsum + col-1023 edge
            vector.tensor_reduce(stag[:, RSP5:RSP5 + 1], m2c[:, 3840:4096],
                                 mybir.AxisListType.X, AluOpType.add,
                                 apply_absolute_value=True)
            vector.tensor_reduce(stag[:, Q3C1:Q3C1 + 1], m2c[:, 4095:4096],
                                 mybir.AxisListType.X, AluOpType.add,
                                 apply_absolute_value=True).then_inc(dve_fin, 1)

        @block.scalar
        def _(scalar):
            def act(col, m2, c0, c1, wait=None, inc=False):
                if wait is not None:
                    scalar.wait_ge(min2_done, wait)
                r = scalar.activation(absout[:, 0:c1 - c0], m2[:, c0:c1],
                                      mybir.ActivationFunctionType.Abs,
                                      bias=zbias[:],
                                      accum_out=stag[:, col:col + 1])
                if inc:
                    r.then_inc(act_done, 1)

            # bulk rowsums, gated per tile; h0 chunks isolate q0 (row 0),
            # h1 chunks isolate q3 (row 1023)
            act(0, m2a, 0, 1024, wait=1, inc=True)       # chunk0 q0
            act(1, m2a, 1024, 2048, inc=True)            # chunk0 q1
            act(2, m2a, 2048, 4096, wait=2, inc=True)    # chunk0 q23
            act(3, m2b, 0, 2048, wait=3)                 # chunk1 q01
            act(4, m2b, 2048, 3072, wait=4)              # chunk1 q2
            act(5, m2b, 3072, 4096)                      # chunk1 q3
            act(6, m2a, 0, 1024, wait=5)                 # chunk2 q0
            act(7, m2a, 1024, 2048)                      # chunk2 q1
            act(EDGE2 + 0, m2a, 0, 1)                    # chunk2 q0c0
            act(EDGE2 + 1, m2a, 1023, 1024)
            act(EDGE2 + 2, m2a, 1024, 1025)
            act(EDGE2 + 3, m2a, 2047, 2048)
            act(8, m2a, 2048, 4096, wait=6)              # chunk2 q23
            act(EDGE2 + 4, m2a, 2048, 2049)
            act(EDGE2 + 5, m2a, 3071, 3072)
            act(EDGE2 + 6, m2a, 3072, 3073)
            act(EDGE2 + 7, m2a, 4095, 4096)
            # piece rowsums
            act(9, m2c, 0, 1024, wait=7)
            act(10, m2c, 1024, 2048, wait=8)
            act(11, m2c, 2048, 3072, wait=9)
            act(12, m2c, 3072, 3584, wait=10)
            act(13, m2c, 3584, 3840, wait=11)
            # accum-retire guard: read the last accum col before trusting
            # any accum landed (walrus splits ACTIVATE/READ_ACCUMULATOR)
            scalar.copy(acksink[:], stag[:, 13:14]).then_inc(act_fin, 1)

    nc.finalize()
    return nc


def make_in_maps(x: np.ndarray) -> list:
    x = np.ascontiguousarray(np.asarray(x, dtype=np.float32))
    return [{"x": x[i * BPC:(i + 1) * BPC].reshape(BPC, C, 2, P, F)}
            for i in range(NCORES)]


def _finish_host(results) -> np.float32:
    total = 0.0
    for r in results:
        a = np.asarray(r["out"], dtype=np.float64)
        cs = a.sum(axis=0)  # per-column partition sums
        for img in range(2):
            if img == 0:
                R = cs[0:6].sum()
                row0, row1023 = a[0, 0], a[127, 5]
                E = cs[EDGE0:EDGE0 + 16].sum()
                e0 = a[0, EDGE0] + a[0, EDGE0 + 1]
                e1023 = a[127, EDGE1 + 6] + a[127, EDGE1 + 7]
            else:
                R = cs[6:15].sum()
                row0 = a[0, 6]
                row1023 = a[127, 12] + a[127, 13] + a[127, RSP5]
                E = cs[EDGE2:NCOLS].sum()
                e0 = a[0, EDGE2] + a[0, EDGE2 + 1]
                e1023 = a[127, Q3C0] + a[127, Q3C1]
            total += 3.0 * (3.0 * R - row0 - row1023) \
                - (3.0 * E - e0 - e1023)
    return np.float32(total)


def kernel(**inputs) -> np.ndarray:
    x = np.asarray(inputs["x"], dtype=np.float32)
    assert x.shape == (B, C, H, W), x.shape
    win = int(np.asarray(inputs.get("win_size", 3)))
    assert win == 3, f"kernel specialized for win_size=3, got {win}"

    if "nc" not in _CACHE:
        _CACHE["nc"] = build_nc()
    nc = _CACHE["nc"]

    res = run_bass_kernel_spmd(nc, make_in_maps(x), list(range(NCORES)))
    return np.array(_finish_host(res.results), dtype=np.float32)
